# revision 26
# baseline (speedup 1.0000x reference)
"""Butterfly block-sparse linear kernel for Trainium2 (8 NeuronCores, SPMD).

Computes: y = blockdiag_butterfly(x, factorL, factorR) + bias
  x:(4,2048,4096) f32, factorL/factorR:(8,512,512) f32, bias:(4096,) f32

Math (reference):
  out1[b,k,q] = sum_p x[b, 512k+p] * factorL[k,q,p]      (8 blocks of 512x512)
  z[b,l,r]    = out1_flat[b, 8r+l]                        (butterfly permute)
  out2[b,l,s] = sum_r z[b,l,r] * factorR[l,s,r]
  y[b, 8s+l]  = out2[b,l,s] + bias[8s+l]

Strategy: data-parallel over the 8192 tokens (1024 tokens/core), factors
replicated. Everything bf16 on device (same 1 cycle/row PE rate as fp32r,
half the HBM traffic; rel err ~4e-3 vs the 2e-2 gate), PSUM accumulation f32.

Fully-aligned butterfly: the stage-1 -> stage-2 permute is absorbed into
host-side weight layouts so NO on-chip data movement crosses partitions:
  - stage-1 channel q' order per (k, qc)-tile puts block l = 2qc+(k%2) at
    PSUM partitions 0:64 and l = 2qc+1-(k%2) at 64:128;
  - z tile j (r-chunk for k=2j,2j+1) stores, for slot l, the k=2j values at
    partitions [64*(l%2), +64) and k=2j+1 at the complement - exactly where
    stage-1 produced them, so every eviction is a same-partition engine copy;
  - stage-2 compensates by half-rolling w2's contraction rows for odd l.
Loop order s1(b0), s1(b1), s2(b0), s2(b1) keeps the PE fed across the
stage-1/2 sync point. All DMA transfers are 128 descriptors x 4KB.
"""

import os
import numpy as np
from contextlib import ExitStack

import ml_dtypes

NCORES = 8
TOK = 8192
TPC = TOK // NCORES          # tokens per core
T = 512                      # tokens per PSUM batch (1 bank)
NB = TPC // T

_CACHE = {}
LAST_RESULT = None


def _build_program():
    import concourse.bacc as bacc
    import concourse.tile as tile
    import concourse.mybir as mybir

    F32 = mybir.dt.float32
    BF16 = mybir.dt.bfloat16
    IDENT = mybir.ActivationFunctionType.Identity

    nc = bacc.Bacc("TRN2", target_bir_lowering=False, debug=False)
    x = nc.dram_tensor("x", [128, 32768], BF16, kind="ExternalInput").ap()
    w1 = nc.dram_tensor("w1", [128, 16384], BF16, kind="ExternalInput").ap()
    w2 = nc.dram_tensor("w2", [128, 16384], BF16, kind="ExternalInput").ap()
    bias = nc.dram_tensor("bias", [128, 32], F32, kind="ExternalInput").ap()
    out = nc.dram_tensor("out", [128, 32768], BF16, kind="ExternalOutput").ap()

    with tile.TileContext(nc) as tc, ExitStack() as ctx:
        wpool = ctx.enter_context(tc.tile_pool(name="w", bufs=1))
        xpool = ctx.enter_context(tc.tile_pool(name="x", bufs=4))
        zpool = ctx.enter_context(tc.tile_pool(name="z", bufs=1))
        opool = ctx.enter_context(tc.tile_pool(name="o", bufs=2))
        ps1 = ctx.enter_context(tc.tile_pool(name="ps1", bufs=4, space="PSUM"))
        ps2 = ctx.enter_context(tc.tile_pool(name="ps2", bufs=4, space="PSUM"))

        bt = wpool.tile([128, 32], F32, tag="bias")
        w1ts = [
            wpool.tile([128, 2048], BF16, name=f"w1_{k}", tag=f"w1_{k}")
            for k in range(8)
        ]
        w2ts = [
            wpool.tile([128, 2048], BF16, name=f"w2_{l}", tag=f"w2_{l}")
            for l in range(8)
        ]
        zts = [
            zpool.tile([128, 4096], BF16, name=f"z_{b}_{j}", tag=f"z_{b}_{j}")
            for b in range(NB)
            for j in range(4)
        ]
        dumw = wpool.tile([128, 128], BF16, tag="dumw")
        nc.vector.memset(dumw[:], 0)

        xloads = {}

        def load_x(b, k, q=None):
            xt = xpool.tile([128, 2048], BF16, tag="xt")
            c0 = (k * 2 + b) * 2048
            (q or nc.gpsimd).dma_start(xt[:], x[:, c0 : c0 + 2048])
            xloads[(b, k)] = xt

        def load_w(wts, wdram, i, q=None):
            (q or nc.gpsimd).dma_start(
                wts[i][:], wdram[:, i * 2048 : (i + 1) * 2048]
            )

        # first pair rides two dedicated HWDGE queues in parallel; the rest
        # of the pipeline streams through the gpsimd SWDGE queue.
        load_x(0, 0, q=nc.sync)
        load_w(w1ts, w1, 0, q=nc.scalar)
        load_x(0, 1)
        load_w(w1ts, w1, 1)
        load_x(0, 2)
        load_w(w1ts, w1, 2)
        nc.gpsimd.dma_start(bt[:], bias[:])

        # warmup: dependency-free matmuls hold the PE at speed until the
        # first real inputs land (~13us; 48 x ~107ns from ~7.3us abuts it),
        # p-state with no ramp. They cycle the ps1 pool (no readers, so
        # the banks recycle immediately).
        for _ in range(48):
            pw = ps1.tile([128, 128], F32, tag="p1")
            nc.tensor.matmul(pw[:], dumw[:], dumw[:], start=True, stop=True)

        S1 = [(b, k) for b in range(NB) for k in range(8)]

        def evict_s1(k, qc, p1, zv):
            lo = k % 2        # l parity at partitions 0:64
            la = 2 * qc + lo
            lb = 2 * qc + 1 - lo
            if qc % 2 == 0:
                nc.vector.tensor_copy(
                    zv[0:64, la * T : (la + 1) * T], p1[0:64, :]
                )
                nc.vector.tensor_copy(
                    zv[64:128, lb * T : (lb + 1) * T], p1[64:128, :]
                )
            else:
                nc.scalar.activation(
                    zv[0:64, la * T : (la + 1) * T], p1[0:64, :], IDENT
                )
                nc.scalar.activation(
                    zv[64:128, lb * T : (lb + 1) * T], p1[64:128, :], IDENT
                )

        for i, (b, k) in enumerate(S1):
            if b == 0 and k + 3 < 8:
                load_w(w1ts, w1, k + 3)
            if i + 3 < len(S1):
                load_x(*S1[i + 3])
            if b == 1:
                # w2 rides the x-only stretch of batch-1 stage 1
                load_w(w2ts, w2, k)
            zv = zts[b * 4 + k // 2]
            xt = xloads.pop((b, k))
            for qc in range(4):
                p1 = ps1.tile([128, T], F32, tag="p1")
                for pc in range(4):
                    col = qc * 512 + pc * 128
                    nc.tensor.matmul(
                        p1[:],
                        w1ts[k][:, col : col + 128],
                        xt[:, pc * T : (pc + 1) * T],
                        start=(pc == 0),
                        stop=(pc == 3),
                    )
                evict_s1(k, qc, p1, zv)

        for b in range(NB):
            for l in range(8):
                last = b == NB - 1 and l == 7
                ot = opool.tile([128, 2048], BF16, tag="ot")
                c0 = (b * 8 + l) * 2048
                for sc in range(4):
                    p2 = ps2.tile([128, T], F32, tag="p2")
                    for j in range(4):
                        col = sc * 512 + j * 128
                        nc.tensor.matmul(
                            p2[:],
                            w2ts[l][:, col : col + 128],
                            zts[b * 4 + j][:, l * T : (l + 1) * T],
                            start=(j == 0),
                            stop=(j == 3),
                        )
                    nc.scalar.activation(
                        ot[:, sc * T : (sc + 1) * T],
                        p2[:],
                        IDENT,
                        bias=bt[:, l * 4 + sc : l * 4 + sc + 1],
                    )
                    if last:
                        # per-sc stores shorten the post-PE tail
                        nc.sync.dma_start(
                            out[:, c0 + sc * T : c0 + (sc + 1) * T],
                            ot[:, sc * T : (sc + 1) * T],
                        )
                if not last:
                    nc.sync.dma_start(out[:, c0 : c0 + 2048], ot[:])
    nc.compile()
    return nc


def _get_program():
    if "nc" not in _CACHE:
        _CACHE["nc"] = _build_program()
    return _CACHE["nc"]


def _ensure_ntff_hook():
    """Bridge the axon NTFF profile hook when the image's antenv lacks it."""
    import sys, types

    try:
        from antenv.axon_hooks import get_axon_ntff_profile_hook  # noqa: F401

        return
    except ImportError:
        pass
    try:
        from trn_agent_boot.trn_boot import _ntff_profile_via_ctypes

        hook = _ntff_profile_via_ctypes("/opt/axon/libaxon_pjrt.so")
        mod = types.ModuleType("antenv.axon_hooks")
        _h = {"hook": hook}
        mod.set_axon_ntff_profile_hook = lambda h: _h.__setitem__("hook", h)
        mod.get_axon_ntff_profile_hook = lambda: _h["hook"]
        sys.modules["antenv.axon_hooks"] = mod
        import antenv

        antenv.axon_hooks = mod
    except Exception:
        pass


def _marshal(x, factorL, factorR, bias):
    """Host-side input marshalling (not device-timed)."""
    BF16 = ml_dtypes.bfloat16

    # x_dev[core][pp, k*4096 + b*2048 + pc*512 + t] = x[token c*1024+b*512+t,
    #   feature 512k+128pc+pp]
    xb = x.reshape(TOK, 4096).astype(BF16)
    xdev = np.ascontiguousarray(
        xb.reshape(NCORES, NB, T, 8, 4, 128).transpose(0, 5, 3, 1, 4, 2)
    ).reshape(NCORES, 128, 32768)

    # stage-1 channel permutation: per (k, qc)-tile, PSUM partition p' holds
    # original channel q: p'<64 -> l=2qc+(k%2), c=p'; p'>=64 -> the other l,
    # c=p'-64; q = 8c + l.
    pprime = np.arange(128)
    ql = np.empty((8, 4, 128), dtype=np.int64)
    for k in range(8):
        for qc in range(4):
            la = 2 * qc + (k % 2)
            lb = 2 * qc + 1 - (k % 2)
            qs = np.where(pprime < 64, 8 * pprime + la, 8 * (pprime - 64) + lb)
            ql[k, qc] = qs
    w1t = factorL.astype(BF16).transpose(0, 2, 1)  # [k, p, q]
    w1dev = np.empty((128, 16384), dtype=BF16)
    for k in range(8):
        tmp = w1t[k][:, ql[k]]                      # [512 p, 4 qc, 128 p']
        tmp = tmp.reshape(4, 128, 4, 128)           # [pc, pp, qc, p']
        w1dev[:, k * 2048 : (k + 1) * 2048] = tmp.transpose(1, 2, 0, 3).reshape(
            128, 2048
        )

    # stage-2 contraction rows: z tile j partition p holds r = 128j+p for even
    # l, r = 128j+(p+64)%128 for odd l.
    p = np.arange(128)
    w2t = factorR.astype(BF16).transpose(0, 2, 1)  # [l, r, s]
    w2dev = np.empty((128, 16384), dtype=BF16)
    for l in range(8):
        rows = (np.arange(4)[:, None] * 128) + (
            p[None, :] if l % 2 == 0 else (p[None, :] + 64) % 128
        )                                           # [j, p]
        tmp = w2t[l][rows]                          # [4 j, 128 p, 512 s]
        tmp = tmp.reshape(4, 128, 4, 128)           # [j, p, sc, si]
        w2dev[:, l * 2048 : (l + 1) * 2048] = tmp.transpose(1, 2, 0, 3).reshape(
            128, 2048
        )

    biasdev = np.ascontiguousarray(
        bias.reshape(4, 128, 8).transpose(1, 2, 0).reshape(128, 32)
    )
    return xdev, w1dev, w2dev, biasdev


def kernel(x, factorL, factorR, bias):
    global LAST_RESULT
    from concourse.bass_utils import run_bass_kernel_spmd

    x = np.asarray(x, dtype=np.float32)
    factorL = np.asarray(factorL, dtype=np.float32)
    factorR = np.asarray(factorR, dtype=np.float32)
    bias = np.asarray(bias, dtype=np.float32)

    xdev, w1dev, w2dev, biasdev = _marshal(x, factorL, factorR, bias)

    in_maps = [
        {"x": xdev[c], "w1": w1dev, "w2": w2dev, "bias": biasdev}
        for c in range(NCORES)
    ]
    nc = _get_program()
    trace = os.environ.get("BUTTERFLY_TRACE", "0") == "1"
    if trace:
        _ensure_ntff_hook()
    LAST_RESULT = run_bass_kernel_spmd(
        nc, in_maps, list(range(NCORES)), trace=trace
    )
    # out_dev[core][si, (b*8+l)*2048 + sc*512 + t] = y[token c*1024+b*512+t,
    #   feature 1024sc+8si+l]
    odev = np.stack(
        [np.asarray(LAST_RESULT.results[c]["out"]) for c in range(NCORES)]
    ).astype(np.float32)
    y = odev.reshape(NCORES, 128, NB, 8, 4, T).transpose(0, 2, 5, 4, 1, 3)
    return np.ascontiguousarray(y).reshape(4, 2048, 4096)


# revision 27
# speedup vs baseline: 1.1891x; 1.1891x over previous
"""Butterfly block-sparse linear kernel for Trainium2 (8 NeuronCores, SPMD).

Computes: y = blockdiag_butterfly(x, factorL, factorR) + bias
  x:(4,2048,4096) f32, factorL/factorR:(8,512,512) f32, bias:(4096,) f32

Math (reference):
  out1[b,k,q] = sum_p x[b, 512k+p] * factorL[k,q,p]      (8 blocks of 512x512)
  z[b,l,r]    = out1_flat[b, 8r+l]                        (butterfly permute)
  out2[b,l,s] = sum_r z[b,l,r] * factorR[l,s,r]
  y[b, 8s+l]  = out2[b,l,s] + bias[8s+l]

Strategy: data-parallel over the 8192 tokens (1024 tokens/core), factors
replicated. Everything bf16 on device (same 1 cycle/row PE rate as fp32r,
half the HBM traffic; rel err ~4e-3 vs the 2e-2 gate), PSUM accumulation f32.

Fully-aligned butterfly: the stage-1 -> stage-2 permute is absorbed into
host-side weight layouts so NO on-chip data movement crosses partitions:
  - stage-1 channel q' order per (k, qc)-tile puts block l = 2qc+(k%2) at
    PSUM partitions 0:64 and l = 2qc+1-(k%2) at 64:128;
  - z tile j (r-chunk for k=2j,2j+1) stores, for slot l, the k=2j values at
    partitions [64*(l%2), +64) and k=2j+1 at the complement - exactly where
    stage-1 produced them, so every eviction is a same-partition engine copy;
  - stage-2 compensates by half-rolling w2's contraction rows for odd l.
Loop order s1(b0), s1(b1), s2(b0), s2(b1) keeps the PE fed across the
stage-1/2 sync point. All DMA transfers are 128 descriptors x 4KB.
"""

import os
import numpy as np
from contextlib import ExitStack

import ml_dtypes

NCORES = 8
TOK = 8192
TPC = TOK // NCORES          # tokens per core
T = 512                      # tokens per PSUM batch (1 bank)
NB = TPC // T

_CACHE = {}
LAST_RESULT = None


def _build_program():
    import concourse.bacc as bacc
    import concourse.tile as tile
    import concourse.mybir as mybir

    F32 = mybir.dt.float32
    BF16 = mybir.dt.bfloat16
    IDENT = mybir.ActivationFunctionType.Identity

    nc = bacc.Bacc("TRN2", target_bir_lowering=False, debug=False)
    x = nc.dram_tensor("x", [128, 32768], BF16, kind="ExternalInput").ap()
    w1 = nc.dram_tensor("w1", [128, 16384], BF16, kind="ExternalInput").ap()
    w2 = nc.dram_tensor("w2", [128, 16384], BF16, kind="ExternalInput").ap()
    bias = nc.dram_tensor("bias", [128, 32], F32, kind="ExternalInput").ap()
    out = nc.dram_tensor("out", [128, 32768], BF16, kind="ExternalOutput").ap()

    with tile.TileContext(nc) as tc, ExitStack() as ctx:
        wpool = ctx.enter_context(tc.tile_pool(name="w", bufs=1))
        xpool = ctx.enter_context(tc.tile_pool(name="x", bufs=4))
        zpool = ctx.enter_context(tc.tile_pool(name="z", bufs=1))
        opool = ctx.enter_context(tc.tile_pool(name="o", bufs=2))
        ps1 = ctx.enter_context(tc.tile_pool(name="ps1", bufs=4, space="PSUM"))
        ps2 = ctx.enter_context(tc.tile_pool(name="ps2", bufs=4, space="PSUM"))

        bt = wpool.tile([128, 32], F32, tag="bias")
        w1ts = [
            wpool.tile([128, 2048], BF16, name=f"w1_{k}", tag=f"w1_{k}")
            for k in range(8)
        ]
        w2ts = [
            wpool.tile([128, 2048], BF16, name=f"w2_{l}", tag=f"w2_{l}")
            for l in range(8)
        ]
        zts = [
            zpool.tile([128, 4096], BF16, name=f"z_{b}_{j}", tag=f"z_{b}_{j}")
            for b in range(NB)
            for j in range(4)
        ]
        dumw = wpool.tile([128, 128], BF16, tag="dumw")
        nc.vector.memset(dumw[:], 0)

        xloads = {}

        def load_x(b, k, q=None):
            xt = xpool.tile([128, 2048], BF16, tag="xt")
            c0 = (k * 2 + b) * 2048
            (q or nc.gpsimd).dma_start(xt[:], x[:, c0 : c0 + 2048])
            xloads[(b, k)] = xt

        def load_w(wts, wdram, i, q=None):
            (q or nc.gpsimd).dma_start(
                wts[i][:], wdram[:, i * 2048 : (i + 1) * 2048]
            )

        # first pair rides two dedicated HWDGE queues in parallel; the rest
        # of the pipeline streams through the gpsimd SWDGE queue.
        load_x(0, 0, q=nc.sync)
        load_w(w1ts, w1, 0, q=nc.scalar)
        load_x(0, 1)
        load_w(w1ts, w1, 1)
        load_x(0, 2)
        load_w(w1ts, w1, 2)
        nc.gpsimd.dma_start(bt[:], bias[:])

        # warmup: dependency-free matmuls hold the PE at speed until the
        # first real inputs land (~13us; 48 x ~107ns from ~7.3us abuts it),
        # p-state with no ramp. They cycle the ps1 pool (no readers, so
        # the banks recycle immediately).
        for _ in range(46):
            pw = ps1.tile([128, 128], F32, tag="p1")
            nc.tensor.matmul(pw[:], dumw[:], dumw[:], start=True, stop=True)
        # fine-grained tail (64-row, ~30-50ns each) abuts the first real
        # matmul's sem-release (~13.2us) with minimal overshoot quantization
        for _ in range(24):
            pw = ps1.tile([128, 64], F32, tag="p1")
            nc.tensor.matmul(
                pw[0:64, :], dumw[:, 0:64], dumw[:, 0:64],
                start=True, stop=True,
            )

        S1 = [(b, k) for b in range(NB) for k in range(8)]

        def evict_s1(k, qc, p1, zv):
            lo = k % 2        # l parity at partitions 0:64
            la = 2 * qc + lo
            lb = 2 * qc + 1 - lo
            if qc % 2 == 0:
                nc.vector.tensor_copy(
                    zv[0:64, la * T : (la + 1) * T], p1[0:64, :]
                )
                nc.vector.tensor_copy(
                    zv[64:128, lb * T : (lb + 1) * T], p1[64:128, :]
                )
            else:
                nc.scalar.activation(
                    zv[0:64, la * T : (la + 1) * T], p1[0:64, :], IDENT
                )
                nc.scalar.activation(
                    zv[64:128, lb * T : (lb + 1) * T], p1[64:128, :], IDENT
                )

        for i, (b, k) in enumerate(S1):
            if b == 0 and k + 3 < 8:
                load_w(w1ts, w1, k + 3)
            if i + 3 < len(S1):
                load_x(*S1[i + 3])
            if b == 1:
                # w2 rides the x-only stretch of batch-1 stage 1
                load_w(w2ts, w2, k)
            zv = zts[b * 4 + k // 2]
            xt = xloads.pop((b, k))
            for qc in range(4):
                p1 = ps1.tile([128, T], F32, tag="p1")
                for pc in range(4):
                    col = qc * 512 + pc * 128
                    nc.tensor.matmul(
                        p1[:],
                        w1ts[k][:, col : col + 128],
                        xt[:, pc * T : (pc + 1) * T],
                        start=(pc == 0),
                        stop=(pc == 3),
                    )
                evict_s1(k, qc, p1, zv)

        for b in range(NB):
            for l in range(8):
                last = b == NB - 1 and l == 7
                ot = opool.tile([128, 2048], BF16, tag="ot")
                c0 = (b * 8 + l) * 2048
                for sc in range(4):
                    p2 = ps2.tile([128, T], F32, tag="p2")
                    for j in range(4):
                        col = sc * 512 + j * 128
                        nc.tensor.matmul(
                            p2[:],
                            w2ts[l][:, col : col + 128],
                            zts[b * 4 + j][:, l * T : (l + 1) * T],
                            start=(j == 0),
                            stop=(j == 3),
                        )
                    nc.scalar.activation(
                        ot[:, sc * T : (sc + 1) * T],
                        p2[:],
                        IDENT,
                        bias=bt[:, l * 4 + sc : l * 4 + sc + 1],
                    )
                    if last:
                        # per-sc stores shorten the post-PE tail
                        nc.sync.dma_start(
                            out[:, c0 + sc * T : c0 + (sc + 1) * T],
                            ot[:, sc * T : (sc + 1) * T],
                        )
                if not last:
                    nc.sync.dma_start(out[:, c0 : c0 + 2048], ot[:])
    nc.compile()
    return nc


def _get_program():
    if "nc" not in _CACHE:
        _CACHE["nc"] = _build_program()
    return _CACHE["nc"]


def _ensure_ntff_hook():
    """Bridge the axon NTFF profile hook when the image's antenv lacks it."""
    import sys, types

    try:
        from antenv.axon_hooks import get_axon_ntff_profile_hook  # noqa: F401

        return
    except ImportError:
        pass
    try:
        from trn_agent_boot.trn_boot import _ntff_profile_via_ctypes

        hook = _ntff_profile_via_ctypes("/opt/axon/libaxon_pjrt.so")
        mod = types.ModuleType("antenv.axon_hooks")
        _h = {"hook": hook}
        mod.set_axon_ntff_profile_hook = lambda h: _h.__setitem__("hook", h)
        mod.get_axon_ntff_profile_hook = lambda: _h["hook"]
        sys.modules["antenv.axon_hooks"] = mod
        import antenv

        antenv.axon_hooks = mod
    except Exception:
        pass


def _marshal(x, factorL, factorR, bias):
    """Host-side input marshalling (not device-timed)."""
    BF16 = ml_dtypes.bfloat16

    # x_dev[core][pp, k*4096 + b*2048 + pc*512 + t] = x[token c*1024+b*512+t,
    #   feature 512k+128pc+pp]
    xb = x.reshape(TOK, 4096).astype(BF16)
    xdev = np.ascontiguousarray(
        xb.reshape(NCORES, NB, T, 8, 4, 128).transpose(0, 5, 3, 1, 4, 2)
    ).reshape(NCORES, 128, 32768)

    # stage-1 channel permutation: per (k, qc)-tile, PSUM partition p' holds
    # original channel q: p'<64 -> l=2qc+(k%2), c=p'; p'>=64 -> the other l,
    # c=p'-64; q = 8c + l.
    pprime = np.arange(128)
    ql = np.empty((8, 4, 128), dtype=np.int64)
    for k in range(8):
        for qc in range(4):
            la = 2 * qc + (k % 2)
            lb = 2 * qc + 1 - (k % 2)
            qs = np.where(pprime < 64, 8 * pprime + la, 8 * (pprime - 64) + lb)
            ql[k, qc] = qs
    w1t = factorL.astype(BF16).transpose(0, 2, 1)  # [k, p, q]
    w1dev = np.empty((128, 16384), dtype=BF16)
    for k in range(8):
        tmp = w1t[k][:, ql[k]]                      # [512 p, 4 qc, 128 p']
        tmp = tmp.reshape(4, 128, 4, 128)           # [pc, pp, qc, p']
        w1dev[:, k * 2048 : (k + 1) * 2048] = tmp.transpose(1, 2, 0, 3).reshape(
            128, 2048
        )

    # stage-2 contraction rows: z tile j partition p holds r = 128j+p for even
    # l, r = 128j+(p+64)%128 for odd l.
    p = np.arange(128)
    w2t = factorR.astype(BF16).transpose(0, 2, 1)  # [l, r, s]
    w2dev = np.empty((128, 16384), dtype=BF16)
    for l in range(8):
        rows = (np.arange(4)[:, None] * 128) + (
            p[None, :] if l % 2 == 0 else (p[None, :] + 64) % 128
        )                                           # [j, p]
        tmp = w2t[l][rows]                          # [4 j, 128 p, 512 s]
        tmp = tmp.reshape(4, 128, 4, 128)           # [j, p, sc, si]
        w2dev[:, l * 2048 : (l + 1) * 2048] = tmp.transpose(1, 2, 0, 3).reshape(
            128, 2048
        )

    biasdev = np.ascontiguousarray(
        bias.reshape(4, 128, 8).transpose(1, 2, 0).reshape(128, 32)
    )
    return xdev, w1dev, w2dev, biasdev


def kernel(x, factorL, factorR, bias):
    global LAST_RESULT
    from concourse.bass_utils import run_bass_kernel_spmd

    x = np.asarray(x, dtype=np.float32)
    factorL = np.asarray(factorL, dtype=np.float32)
    factorR = np.asarray(factorR, dtype=np.float32)
    bias = np.asarray(bias, dtype=np.float32)

    xdev, w1dev, w2dev, biasdev = _marshal(x, factorL, factorR, bias)

    in_maps = [
        {"x": xdev[c], "w1": w1dev, "w2": w2dev, "bias": biasdev}
        for c in range(NCORES)
    ]
    nc = _get_program()
    trace = os.environ.get("BUTTERFLY_TRACE", "0") == "1"
    if trace:
        _ensure_ntff_hook()
    LAST_RESULT = run_bass_kernel_spmd(
        nc, in_maps, list(range(NCORES)), trace=trace
    )
    # out_dev[core][si, (b*8+l)*2048 + sc*512 + t] = y[token c*1024+b*512+t,
    #   feature 1024sc+8si+l]
    odev = np.stack(
        [np.asarray(LAST_RESULT.results[c]["out"]) for c in range(NCORES)]
    ).astype(np.float32)
    y = odev.reshape(NCORES, 128, NB, 8, 4, T).transpose(0, 2, 5, 4, 1, 3)
    return np.ascontiguousarray(y).reshape(4, 2048, 4096)
